# revision 32
# baseline (speedup 1.0000x reference)
"""Multi-head attention (B=2, S=2048, D=768, H=12) on 8 trn2 NeuronCores.

Sharding: batch x head-group data/tensor parallel. Core c = b*4+g handles
batch b and heads [3g, 3g+3) (a 192-wide slice of the QKV projections and
the matching 192-row slice of Wo). Each core emits a partial [2048, 768]
fp16 output; the host sums the 4 head-group partials per batch and adds bo.

Device schedule. The kernel is dual-roofline (~100us PE streaming, ~97us
ACT exp). The DMA engines sustain ~330GB/s only with large contiguous
lines (they are descriptor-bound at ~10ns/partition-line), so inputs are
host-packed:
  wA  [128, 2312]   biases(f16) | wk | wq            (4.6KB lines)
  xqP [128, 12288]  column halves x chunks x 1024    (4KB-line pair xfers)
  xkP [128, 12288]  d-major chunks x 2048            (8KB-line pair xfers)
  wB  [128, 2688]   wv | wo_a | wo_b mirrored        (5.4KB lines)
  xvR [128, 12288]  seq-tile-quad groups             (6KB lines)
DMA order: wA, xq-half0 (3 chunk-pair transfers), xk (3 pair transfers),
wB, xv groups, xq-half1. Projections consume each pair as it lands; the
first exp fires ~23us in. Warmup junk matmuls bridge the DMA wait so the
PE HAM clock gate stays open.

Only k passA and q passA columns 0:1024 run before attention. The rest
(q passA cols 1024:2048, k/q passB) is deferred into attention PE slack
as six-matmul units cycling through the 2-bank "w" PSUM pool; the v
projection runs just-in-time per seq tile inside qt0.

The attention itself is ONE flat software-pipelined stream over phases
h01(qt0..3) then h2(qt0..3): at every step the NEXT step's score matmuls
are emitted before this step's ctx matmuls — across phase boundaries too
— so the scalar engine's monotonic semaphore wait for exp[i+1] never
covers ctx[i] and the exp stream never drains at a boundary. h2 phases
carry the previous q-tile's output projection, one (st, ns) unit per
iteration; out_proj(qt3) is the tail (borrowing idle score banks so four
units are in flight, copies alternating Scalar/Vector). normalize()
copies the accumulator to SBUF immediately so the PSUM bank frees early;
ctxT_b and wo_b are mirrored into partitions 64:127 so out_proj b-matmuls
alternate PE row groups.
"""

import numpy as np

D_MODEL = 768
NUM_HEADS = 12
D_K = 64
B = 2
S = 2048
N_CORES = 8
G = 4              # head groups (cores per batch)
GW = D_MODEL // G  # 192 features per group = 3 heads
HPG = 3            # heads per group
DC = D_MODEL // 128  # 6 d_model chunks
QT = 512           # q-tile width
NQT = S // QT      # 4
KC = S // 128      # 16 k chunks
ST = S // 128      # 16 seq tiles
BPK = 8            # packed bias columns
WA = BPK + 2 * DC * GW        # 2312: bias | wk | wq
WB = DC * GW + 2 * D_MODEL    # 2688: wv | wo_a | wo_b(mirrored)
XH = DC * 1024                # 6144: one xq half (6 chunks x 1024 seq)
XVW = 4 * DC * 128            # 3072: one xv group (4 seq tiles)

_PROGRAM = None


def _build_program():
    from concourse import bacc, tile
    import concourse.mybir as mybir

    f16 = mybir.dt.float16
    f32 = mybir.dt.float32
    Exp = mybir.ActivationFunctionType.Exp
    mult = mybir.AluOpType.mult

    nc = bacc.Bacc("TRN2", target_bir_lowering=False, debug=False,
                   enable_asserts=False)

    xkP = nc.dram_tensor("xkP", [128, DC * S], f16, kind="ExternalInput")
    xqP = nc.dram_tensor("xqP", [128, 2 * XH], f16, kind="ExternalInput")
    xvR = nc.dram_tensor("xvR", [128, 4 * XVW], f16, kind="ExternalInput")
    wA = nc.dram_tensor("wA", [128, WA], f16, kind="ExternalInput")
    wB = nc.dram_tensor("wB", [128, WB], f16, kind="ExternalInput")
    # partition-major output: out[p, st*768 + c] = result[st*128 + p, c].
    # One DMA per seq-tile PAIR with 3KB lines (the DMA engines are
    # descriptor-bound, so fewer/larger lines beat the row-major layout).
    out = nc.dram_tensor("out", [128, ST * D_MODEL], f16,
                         kind="ExternalOutput")

    with tile.TileContext(nc) as tc:
        with tc.tile_pool(name="const", bufs=1) as cp, \
             tc.tile_pool(name="expp", bufs=6) as ep, \
             tc.tile_pool(name="normp", bufs=3) as np_, \
             tc.tile_pool(name="outp", bufs=3) as op, \
             tc.tile_pool(name="ps_s", bufs=2, space="PSUM") as ps_s, \
             tc.tile_pool(name="ps_c", bufs=2, space="PSUM") as ps_c, \
             tc.tile_pool(name="ps_w", bufs=2, space="PSUM") as ps_w:

            # ---- DMA ----
            wa = cp.tile([128, WA], f16, name="wa")
            nc.sync.dma_start(out=wa[:], in_=wA[:])
            wk_sb = [wa[:, BPK + d * GW:BPK + (d + 1) * GW]
                     for d in range(DC)]
            wq_sb = [wa[:, BPK + DC * GW + d * GW:
                        BPK + DC * GW + (d + 1) * GW] for d in range(DC)]

            xq_sb = cp.tile([128, 2 * XH], f16, name="xq_sb")
            for p3 in range(3):
                nc.sync.dma_start(
                    out=xq_sb[:, p3 * 2048:(p3 + 1) * 2048],
                    in_=xqP[:, p3 * 2048:(p3 + 1) * 2048])
            xk_sb = cp.tile([128, DC * S], f16, name="xk_sb")
            for p3 in range(3):
                nc.sync.dma_start(
                    out=xk_sb[:, p3 * 4096:(p3 + 1) * 4096],
                    in_=xkP[:, p3 * 4096:(p3 + 1) * 4096])

            wb = cp.tile([128, WB], f16, name="wb")
            nc.sync.dma_start(out=wb[:], in_=wB[:])
            wv_sb = [wb[:, d * GW:(d + 1) * GW] for d in range(DC)]
            wo_a = wb[:, DC * GW:DC * GW + D_MODEL]
            wo_bm = wb[:, DC * GW + D_MODEL:WB]

            # all xv groups before xq-half1: qt0's just-in-time v projection
            # paces the early exp stream; xq cols 1024:2048 are only needed
            # by the deferred q passA units in qt1.
            xvg = [cp.tile([128, XVW], f16, name=f"xv{g}")
                   for g in range(4)]
            for g in range(4):
                nc.sync.dma_start(out=xvg[g][:],
                                  in_=xvR[:, g * XVW:(g + 1) * XVW])
            nc.sync.dma_start(out=xq_sb[:, XH:2 * XH],
                              in_=xqP[:, XH:2 * XH])

            def xk_v(d, c0, w):
                return xk_sb[:, d * 2048 + c0:d * 2048 + c0 + w]

            def xq_v(d, c0, w):
                h, c1 = divmod(c0, 1024)
                return xq_sb[:, h * XH + d * 1024 + c1:
                             h * XH + d * 1024 + c1 + w]

            # biases as f32 scalars (wa holds them as f16)
            bps = cp.tile([128, BPK], f32, name="bps")
            nc.vector.tensor_copy(out=bps[:], in_=wa[:, 0:BPK])
            bq_a, bq_b = bps[:, 0:1], bps[0:64, 1:2]
            bk_a, bk_b = bps[:, 2:3], bps[0:64, 3:4]
            bv_h = [bps[0:64, 4 + h:5 + h] for h in range(HPG)]

            # ---- PE warmup bridging the DMA wait ----
            junk = cp.tile([128, QT], f16, name="junk")
            nc.vector.memset(junk[:], 0.5)
            wupt = [ps_w.tile([128, QT], f32, name="W", tag="w")
                    for _ in range(2)]
            for i in range(24):
                nc.tensor.matmul(wupt[i % 2][:], lhsT=junk[:, 0:128],
                                 rhs=junk[:], start=True, stop=True)
            # dummy exp pulls the ACT exp-table load into the DMA shadow
            escr = cp.tile([128, BPK], f16, name="escr")
            nc.scalar.activation(escr[:], bps[:], Exp, scale=0.001)

            # ---- pre-attention projections, consuming chunk pairs as
            # they land: q passA cols 0:1024, then k passA (all cols) ----
            qT_a = cp.tile([128, S], f16, name="qT_a")
            qT_b = cp.tile([128, S], f16, name="qT_b")
            kT_a = cp.tile([128, S], f16, name="kT_a")
            kT_b = cp.tile([128, S], f16, name="kT_b")

            pjQ = ps_s.tile([128, 2 * QT], f32, name="S", tag="s")
            for d in range(DC):
                for n in range(2):
                    nc.tensor.matmul(
                        pjQ[:, n * QT:(n + 1) * QT],
                        lhsT=wq_sb[d][:, 0:128], rhs=xq_v(d, n * QT, QT),
                        start=(d == 0), stop=(d == DC - 1))
            for n in range(2):
                nc.vector.tensor_scalar_add(
                    qT_a[:, n * QT:(n + 1) * QT],
                    pjQ[:, n * QT:(n + 1) * QT], bq_a)

            pjK = [ps_s.tile([128, 2 * QT], f32, name="S", tag="s")
                   for _ in range(2)]
            for d in range(DC):
                for j2 in range(2):
                    for n in range(2):
                        nc.tensor.matmul(
                            pjK[j2][:, n * QT:(n + 1) * QT],
                            lhsT=wk_sb[d][:, 0:128],
                            rhs=xk_v(d, j2 * 1024 + n * QT, QT),
                            start=(d == 0), stop=(d == DC - 1))
            for j2 in range(2):
                for n in range(2):
                    cs = slice(j2 * 1024 + n * QT, j2 * 1024 + (n + 1) * QT)
                    nc.vector.tensor_scalar_add(
                        kT_a[:, cs], pjK[j2][:, n * QT:(n + 1) * QT], bk_a)

            # ---- deferred projection units ----
            fillers = []

            def add_unit(xv_fn, wsel, w_lo, w_hi, b, dst, c0, last, mirror):
                state = {}
                rows = w_hi - w_lo

                def mk(d):
                    def emit():
                        if d == 0:
                            state["pj"] = ps_w.tile([128, QT], f32,
                                                    name="W", tag="w")
                        nc.tensor.matmul(
                            state["pj"][0:rows, :],
                            lhsT=wsel[d][:, w_lo:w_hi],
                            rhs=xv_fn(d, c0, QT),
                            start=(d == 0), stop=(d == DC - 1))
                        if d == DC - 1:
                            nc.vector.tensor_scalar_add(
                                dst[0:rows, c0:c0 + QT],
                                state["pj"][0:rows, :], b)
                            if mirror:
                                nc.sync.dma_start(
                                    out=dst[64:128, c0:c0 + QT],
                                    in_=dst[0:64, c0:c0 + QT])
                    return emit
                for d in range(DC):
                    fillers.append(mk(d))

            for n in range(2):  # q passA cols 1024:2048
                add_unit(xq_v, wq_sb, 0, 128, bq_a, qT_a,
                         1024 + n * QT, False, False)
            for n4 in range(4):  # k passB
                add_unit(xk_v, wk_sb, 128, GW, bk_b, kT_b,
                         n4 * QT, n4 == 3, True)
            for n4 in range(4):  # q passB
                add_unit(xq_v, wq_sb, 128, GW, bq_b, qT_b,
                         n4 * QT, n4 == 3, True)

            # ---- v projection: just-in-time per seq tile inside qt0 ----
            v_sb = [None] * ST

            def v_proj(st):
                g4, s4 = st // 4, st % 4
                pv = ps_w.tile([128, QT], f32, name="W", tag="w")
                for d in range(DC):
                    c0 = (s4 * DC + d) * 128
                    nc.tensor.matmul(pv[:, 0:GW],
                                     lhsT=xvg[g4][:, c0:c0 + 128],
                                     rhs=wv_sb[d][:],
                                     start=(d == 0), stop=(d == DC - 1))
                vt = cp.tile([128, HPG, D_K + 1], f16, name=f"vsb{st}")
                nc.vector.tensor_copy(out=vt[:, :, 0:D_K],
                                      in_=pv[:, 0:GW].rearrange(
                                          "p (h w) -> p h w", h=HPG))
                nc.vector.memset(vt[:, :, D_K:D_K + 1], 1.0)
                v_sb[st] = vt

            # ---- attention state ----
            ctxT_a = [cp.tile([128, QT], f16, name=f"ctxTa{j}")
                      for j in range(NQT)]
            ctxT_b = [cp.tile([128, QT], f16, name=f"ctxTb{j}")
                      for j in range(NQT)]

            ones1 = cp.tile([1, D_K], f16, name="ones1")
            nc.vector.memset(ones1[:], 1.0)

            def normalize(C, h, qt):
                # Copy the accumulator (and denominator row) to SBUF first
                # so the PSUM bank frees early. h1's copy lands at
                # partitions 64:128 so the multiply's SBUF operands share a
                # start partition. The final (tail) normalize broadcasts the
                # reciprocal via a PE ones-matmul instead of GPSIMD — the PE
                # is idle there and it keeps the clock gate open.
                base = 64 if h == 1 else 0
                tail = h == 2 and qt == NQT - 1
                ctx_dst = (ctxT_a[qt][0:64] if h == 0 else
                           ctxT_a[qt][64:128] if h == 1 else
                           ctxT_b[qt][0:64])
                den = np_.tile([1, QT], f32, name="den", tag="den")
                if tail:
                    # scalar engine is idle after the last exp; keep the
                    # vector queue clear for the reciprocal chain
                    nc.scalar.copy(den[:], C[D_K:D_K + 1, :])
                else:
                    nc.vector.tensor_copy(out=den[:], in_=C[D_K:D_K + 1, :])
                Cc = np_.tile([128, QT], f32, name="Cc", tag="cc")
                nc.vector.tensor_copy(out=Cc[base:base + D_K, :],
                                      in_=C[0:D_K, :])
                r16 = np_.tile([1, QT], f16, name="r16", tag="r16")
                r = np_.tile([1, QT], f32, name="r", tag="r")
                nc.vector.reciprocal_approx_fast(out=r[:], in_=den[:])
                if tail:
                    nc.vector.tensor_copy(out=r16[:], in_=r[:])
                    bcp = ps_c.tile([128, QT], f32, name="C", tag="c")
                    nc.tensor.matmul(bcp[0:D_K, :], lhsT=ones1[:],
                                     rhs=r16[:], start=True, stop=True)
                    bc_ap = bcp[0:D_K, :]
                else:
                    bc = np_.tile([128, QT], f32, name="bc", tag="bc")
                    nc.gpsimd.partition_broadcast(bc[:], r[:])
                    bc_ap = bc[base:base + D_K, :]
                nc.vector.tensor_tensor(out=ctx_dst[:],
                                        in0=Cc[base:base + D_K, :],
                                        in1=bc_ap,
                                        op=mult)
                nc.vector.tensor_scalar_add(ctx_dst[:], ctx_dst[:], bv_h[h])
                if h == 2 and not tail:
                    # mirror so out_proj b-matmuls can alternate row groups
                    nc.sync.dma_start(out=ctxT_b[qt][64:128, :],
                                      in_=ctxT_b[qt][0:64, :])

            osb2 = [None]  # current [128, 1536] tile covering an st pair

            def ou_a(qt, u, po):
                st = u // 2
                ws = slice(st * 128, (st + 1) * 128)
                ns = slice((u % 2) * 384, (u % 2) * 384 + 384)
                nc.tensor.matmul(po[:], lhsT=ctxT_a[qt][:, ws],
                                 rhs=wo_a[:, ns], start=True, stop=False)

            def ou_b(qt, u, po, tail, sc=False):
                st = u // 2
                ws = slice(st * 128, (st + 1) * 128)
                ns = slice((u % 2) * 384, (u % 2) * 384 + 384)
                rb = (slice(0, 64) if (u % 2 == 0 or tail)
                      else slice(64, 128))
                nc.tensor.matmul(po[:], lhsT=ctxT_b[qt][rb, ws],
                                 rhs=wo_bm[rb, ns], start=False, stop=True)
                oc = (u % 4) * 384
                if sc or (tail and u % 2 == 1):
                    nc.scalar.copy(osb2[0][:, oc:oc + 384], po[:])
                else:
                    nc.vector.tensor_copy(out=osb2[0][:, oc:oc + 384],
                                          in_=po[:])
                if u % 4 == 3:
                    c0 = (qt * 4 + (u // 4) * 2) * D_MODEL
                    nc.sync.dma_start(out=out[:, c0:c0 + 2 * D_MODEL],
                                      in_=osb2[0][:])

            def out_unit(qt, u, po=None, tail=False, sc=False):
                if u % 4 == 0:
                    osb2[0] = op.tile([128, 2 * D_MODEL], f16, name="osb")
                if po is None:
                    po = ps_w.tile([128, QT], f32, name="W",
                                   tag="w")[:, 0:384]
                ou_a(qt, u, po)
                ou_b(qt, u, po, tail, sc)

            # ---- the flat attention stream ----
            # phase descriptors: ("h01", qt) x4 then ("h2", qt) x4
            steps = []
            for qt in range(NQT):
                steps += [("h01", qt, kc) for kc in range(KC)]
            for qt in range(NQT):
                steps += [("h2", qt, kc2) for kc2 in range(KC // 2)]

            Cs = {}   # (kind, qt) -> accumulator tile(s)
            S2q = [None] * len(steps)
            tail_po = []

            def emit_scores(i):
                kind, qt, kc = steps[i]
                S2 = ps_s.tile([128, 2 * QT], f32, name="S", tag="s")
                qs = slice(qt * QT, (qt + 1) * QT)
                if kind == "h01":
                    ks = slice(kc * 128, (kc + 1) * 128)
                    nc.tensor.matmul(S2[:, 0:QT], lhsT=kT_a[0:64, ks],
                                     rhs=qT_a[0:64, qs])
                    nc.tensor.matmul(S2[:, QT:2 * QT],
                                     lhsT=kT_a[64:128, ks],
                                     rhs=qT_a[64:128, qs])
                else:
                    for ii in (0, 1):
                        kcc = 2 * kc + ii
                        rg = slice(64 * ii, 64 * ii + 64)
                        nc.tensor.matmul(
                            S2[:, ii * QT:(ii + 1) * QT],
                            lhsT=kT_b[rg, kcc * 128:(kcc + 1) * 128],
                            rhs=qT_b[rg, qs])
                S2q[i] = S2

            emit_scores(0)
            for i, (kind, qt, kc) in enumerate(steps):
                e2 = ep.tile([128, 2 * QT], f16, name="expT")
                nc.scalar.activation(e2[:], S2q[i][:], Exp, scale=0.125)
                S2q[i] = None
                if i + 1 < len(steps):
                    emit_scores(i + 1)
                # PE-slack extras
                if kind == "h01":
                    if qt == 0:
                        v_proj(kc)
                    else:
                        rate = 2 if (qt == 1 and kc < 4) else 1
                        for _ in range(rate):
                            if fillers:
                                fillers.pop(0)()
                else:
                    if qt > 0:
                        # the last two injected units copy via the (by then
                        # idle) scalar engine so the vector queue is clear
                        # for the tail normalize
                        out_unit(qt - 1, kc,
                                 sc=(qt == NQT - 1 and kc >= 6))
                    elif fillers:
                        fillers.pop(0)()
                # ctx
                if kind == "h01":
                    if kc == 0:
                        Cs[qt] = {h: ps_c.tile([128, QT], f32, name="C",
                                               tag="c") for h in (0, 1)}
                    for h in (0, 1):
                        nc.tensor.matmul(Cs[qt][h][0:D_K + 1, :],
                                         lhsT=v_sb[kc][:, h, :],
                                         rhs=e2[:, h * QT:(h + 1) * QT],
                                         start=(kc == 0), stop=(kc == KC - 1))
                    if kc == KC - 1:
                        for h in (0, 1):
                            normalize(Cs[qt][h], h, qt)
                else:
                    if kc == 0:
                        Cs[("h2", qt)] = ps_c.tile([128, QT], f32,
                                                   name="C", tag="c")
                    C2 = Cs[("h2", qt)]
                    for ii in (0, 1):
                        kcc = 2 * kc + ii
                        nc.tensor.matmul(C2[0:D_K + 1, :],
                                         lhsT=v_sb[kcc][:, 2, :],
                                         rhs=e2[:, ii * QT:(ii + 1) * QT],
                                         start=(kcc == 0),
                                         stop=(kcc == KC - 1))
                    if kc == KC // 2 - 1:
                        if qt == NQT - 1:
                            # prestage the tail out_proj's ctxT_a halves:
                            # they only need ctxT_a, so they run during the
                            # normalize chain, keeping the PE warm. Borrow
                            # the idle score banks + w + c pools so seven
                            # accumulators are in flight.
                            tts = [ps_s.tile([128, 2 * QT], f32, name="S",
                                             tag="s") for _ in range(2)]
                            tail_po.extend([
                                tts[0][:, 0:384], tts[0][:, QT:QT + 384],
                                tts[1][:, 0:384], tts[1][:, QT:QT + 384]])
                            for _ in range(2):
                                tail_po.append(ps_w.tile(
                                    [128, QT], f32, name="W",
                                    tag="w")[:, 0:384])
                            tail_po.append(ps_c.tile(
                                [128, QT], f32, name="C", tag="c")[:, 0:384])
                            for u in range(7):
                                ou_a(NQT - 1, u, tail_po[u])
                        normalize(C2, 2, qt)

            # ---- tail: qt3's out_proj b-halves + copies + DMAs ----
            for u in range(4):
                if u == 0:
                    osb2[0] = op.tile([128, 2 * D_MODEL], f16, name="osb")
                ou_b(NQT - 1, u, tail_po[u], True)
            osb2[0] = op.tile([128, 2 * D_MODEL], f16, name="osb")
            ou_b(NQT - 1, 4, tail_po[4], True)
            ou_b(NQT - 1, 5, tail_po[5], True)
            p7 = ps_w.tile([128, QT], f32, name="W", tag="w")[:, 0:384]
            ou_a(NQT - 1, 7, p7)
            ou_b(NQT - 1, 6, tail_po[6], True)
            ou_b(NQT - 1, 7, p7, True)

    nc.compile()
    return nc


def _get_program():
    global _PROGRAM
    if _PROGRAM is None:
        _PROGRAM = _build_program()
    return _PROGRAM


def make_in_maps(query, key, value, Wq, bq, Wk, bk, Wv, bv, Wo, bo):
    """Build the 8 per-core input maps (host-side shard + pack + cast)."""
    q32 = np.asarray(query, np.float32)
    k32 = np.asarray(key, np.float32)
    v32 = np.asarray(value, np.float32)

    def pack_q(xT):
        # [768, 2048] -> [128, 2*6144]: halves x chunks x 1024
        return np.ascontiguousarray(
            xT.reshape(DC, 128, 2, 1024).transpose(2, 1, 0, 3)
        ).reshape(2, 128, XH).transpose(1, 0, 2).reshape(128, 2 * XH)

    def pack_k(xT):
        # [768, 2048] -> [128, 6*2048]: d-major full-width chunks
        return np.ascontiguousarray(
            xT.reshape(DC, 128, S).transpose(1, 0, 2)).reshape(128, DC * S)

    def pack_v(xT):
        # [768, 2048] -> [128, 16*768]: seq-tile-major
        return np.ascontiguousarray(
            xT.reshape(DC, 128, ST, 128).transpose(1, 2, 0, 3)
        ).reshape(128, ST * DC * 128)

    xP = {}
    for b in range(B):
        xP[b] = (pack_q(q32[b].T.astype(np.float16)),
                 pack_k(k32[b].T.astype(np.float16)),
                 pack_v(v32[b].T.astype(np.float16)))
    Wq = np.asarray(Wq, np.float32)
    Wk = np.asarray(Wk, np.float32)
    Wv = np.asarray(Wv, np.float32)
    Wo = np.asarray(Wo, np.float32)
    bq = np.asarray(bq, np.float32)
    bk = np.asarray(bk, np.float32)
    bv = np.asarray(bv, np.float32)
    in_maps = []
    for c in range(N_CORES):
        b, g = divmod(c, G)
        fs = slice(g * GW, (g + 1) * GW)
        xq, xk, xv = xP[b]
        wa = np.zeros((128, WA), np.float16)
        wa[:, 0] = bq[fs][0:128]
        wa[0:64, 1] = bq[fs][128:GW]
        wa[:, 2] = bk[fs][0:128]
        wa[0:64, 3] = bk[fs][128:GW]
        for h in range(HPG):
            wa[0:64, 4 + h] = bv[fs][h * 64:(h + 1) * 64]
        for i, W in enumerate((Wk, Wq)):
            Ws = W[:, fs]
            for d in range(DC):
                c0 = BPK + (i * DC + d) * GW
                wa[:, c0:c0 + GW] = Ws[d * 128:(d + 1) * 128, :].astype(
                    np.float16)
        wbp = np.zeros((128, WB), np.float16)
        Ws = Wv[:, fs]
        for d in range(DC):
            wbp[:, d * GW:(d + 1) * GW] = \
                Ws[d * 128:(d + 1) * 128, :].astype(np.float16)
        Wos = Wo[fs, :]
        wbp[:, DC * GW:DC * GW + D_MODEL] = Wos[0:128, :].astype(np.float16)
        wob = Wos[128:GW, :].astype(np.float16)
        wbp[0:64, DC * GW + D_MODEL:WB] = wob
        wbp[64:128, DC * GW + D_MODEL:WB] = wob
        in_maps.append({
            "xqP": xq, "xkP": xk, "xvR": xv,
            "wA": wa, "wB": wbp,
        })
    return in_maps


def unpack_out(o2):
    """[128, 16*768] partition-major partial -> [2048, 768]."""
    return np.asarray(o2, np.float32).reshape(
        128, ST, D_MODEL).transpose(1, 0, 2).reshape(S, D_MODEL)


def combine_outputs(results, bo):
    """Sum the per-core partial outputs into the full [B, S, D] output."""
    bo = np.asarray(bo, np.float32)
    out = np.zeros((B, S, D_MODEL), np.float32)
    for c in range(N_CORES):
        b = c // G
        out[b] += unpack_out(results[c]["out"])
    out += bo[None, None, :]
    return out


def kernel(**inputs):
    from concourse.bass_utils import run_bass_kernel_spmd

    nc = _get_program()
    in_maps = make_in_maps(**inputs)
    res = run_bass_kernel_spmd(nc, in_maps, list(range(N_CORES)))
    return combine_outputs(res.results, inputs["bo"])


# revision 33
# speedup vs baseline: 1.1637x; 1.1637x over previous
"""Multi-head attention (B=2, S=2048, D=768, H=12) on 8 trn2 NeuronCores.

Sharding: batch x head-group data/tensor parallel. Core c = b*4+g handles
batch b and heads [3g, 3g+3) (a 192-wide slice of the QKV projections and
the matching 192-row slice of Wo). Each core emits a partial [2048, 768]
fp16 output; the host sums the 4 head-group partials per batch and adds bo.

Device schedule. The kernel is dual-roofline (~100us PE streaming, ~97us
ACT exp). The DMA engines sustain ~330GB/s only with large contiguous
lines (they are descriptor-bound at ~10ns/partition-line), so inputs are
host-packed:
  wA  [128, 2312]   biases(f16) | wk | wq            (4.6KB lines)
  xqP [128, 12288]  column halves x chunks x 1024    (4KB-line pair xfers)
  xkP [128, 12288]  d-major chunks x 2048            (8KB-line pair xfers)
  wB  [128, 2688]   wv | wo_a | wo_b mirrored        (5.4KB lines)
  xvR [128, 12288]  seq-tile-quad groups             (6KB lines)
DMA order: wA, xq-half0 (3 chunk-pair transfers), xk (3 pair transfers),
wB, xv groups, xq-half1. Projections consume each pair as it lands; the
first exp fires ~23us in. Warmup junk matmuls bridge the DMA wait so the
PE HAM clock gate stays open.

Only k passA and q passA columns 0:1024 run before attention. The rest
(q passA cols 1024:2048, k/q passB) is deferred into attention PE slack
as six-matmul units cycling through the 2-bank "w" PSUM pool; the v
projection runs just-in-time per seq tile inside qt0.

The attention itself is ONE flat software-pipelined stream over phases
h01(qt0..3) then h2(qt0..3): at every step the NEXT step's score matmuls
are emitted before this step's ctx matmuls — across phase boundaries too
— so the scalar engine's monotonic semaphore wait for exp[i+1] never
covers ctx[i] and the exp stream never drains at a boundary. h2 phases
carry the previous q-tile's output projection, one (st, ns) unit per
iteration; out_proj(qt3) is the tail (borrowing idle score banks so four
units are in flight, copies alternating Scalar/Vector). normalize()
copies the accumulator to SBUF immediately so the PSUM bank frees early;
ctxT_b and wo_b are mirrored into partitions 64:127 so out_proj b-matmuls
alternate PE row groups.
"""

import numpy as np

D_MODEL = 768
NUM_HEADS = 12
D_K = 64
B = 2
S = 2048
N_CORES = 8
G = 4              # head groups (cores per batch)
GW = D_MODEL // G  # 192 features per group = 3 heads
HPG = 3            # heads per group
DC = D_MODEL // 128  # 6 d_model chunks
QT = 512           # q-tile width
NQT = S // QT      # 4
KC = S // 128      # 16 k chunks
ST = S // 128      # 16 seq tiles
BPK = 8            # packed bias columns
WA = BPK + 2 * DC * GW        # 2312: bias | wk | wq
WB = DC * GW + 2 * D_MODEL    # 2688: wv | wo_a | wo_b(mirrored)
XH = DC * 1024                # 6144: one xq half (6 chunks x 1024 seq)
XVW = 4 * DC * 128            # 3072: one xv group (4 seq tiles)

_PROGRAM = None


def _build_program():
    from concourse import bacc, tile
    import concourse.mybir as mybir

    f16 = mybir.dt.float16
    f32 = mybir.dt.float32
    Exp = mybir.ActivationFunctionType.Exp
    mult = mybir.AluOpType.mult

    nc = bacc.Bacc("TRN2", target_bir_lowering=False, debug=False,
                   enable_asserts=False)

    xkP = nc.dram_tensor("xkP", [128, DC * S], f16, kind="ExternalInput")
    xqP = nc.dram_tensor("xqP", [128, 2 * XH], f16, kind="ExternalInput")
    xvR = nc.dram_tensor("xvR", [128, 4 * XVW], f16, kind="ExternalInput")
    wA = nc.dram_tensor("wA", [128, WA], f16, kind="ExternalInput")
    wB = nc.dram_tensor("wB", [128, WB], f16, kind="ExternalInput")
    # partition-major output: out[p, st*768 + c] = result[st*128 + p, c].
    # One DMA per seq-tile PAIR with 3KB lines (the DMA engines are
    # descriptor-bound, so fewer/larger lines beat the row-major layout).
    out = nc.dram_tensor("out", [128, ST * D_MODEL], f16,
                         kind="ExternalOutput")

    with tile.TileContext(nc) as tc:
        with tc.tile_pool(name="const", bufs=1) as cp, \
             tc.tile_pool(name="expp", bufs=4) as ep, \
             tc.tile_pool(name="normp", bufs=2) as np_, \
             tc.tile_pool(name="outp", bufs=3) as op, \
             tc.tile_pool(name="ps_s", bufs=2, space="PSUM") as ps_s, \
             tc.tile_pool(name="ps_c", bufs=2, space="PSUM") as ps_c, \
             tc.tile_pool(name="ps_w", bufs=2, space="PSUM") as ps_w:

            # ---- DMA ----
            wa = cp.tile([128, WA], f16, name="wa")
            nc.sync.dma_start(out=wa[:], in_=wA[:])
            wk_sb = [wa[:, BPK + d * GW:BPK + (d + 1) * GW]
                     for d in range(DC)]
            wq_sb = [wa[:, BPK + DC * GW + d * GW:
                        BPK + DC * GW + (d + 1) * GW] for d in range(DC)]

            xq_sb = cp.tile([128, 2 * XH], f16, name="xq_sb")
            for p3 in range(3):
                nc.sync.dma_start(
                    out=xq_sb[:, p3 * 2048:(p3 + 1) * 2048],
                    in_=xqP[:, p3 * 2048:(p3 + 1) * 2048])
            xk_sb = cp.tile([128, DC * S], f16, name="xk_sb")
            for p3 in range(3):
                nc.sync.dma_start(
                    out=xk_sb[:, p3 * 4096:(p3 + 1) * 4096],
                    in_=xkP[:, p3 * 4096:(p3 + 1) * 4096])

            wb = cp.tile([128, WB], f16, name="wb")
            nc.sync.dma_start(out=wb[:], in_=wB[:])
            wv_sb = [wb[:, d * GW:(d + 1) * GW] for d in range(DC)]
            wo_a = wb[:, DC * GW:DC * GW + D_MODEL]
            wo_bm = wb[:, DC * GW + D_MODEL:WB]

            # all xv groups before xq-half1: qt0's just-in-time v projection
            # paces the early exp stream; xq cols 1024:2048 are only needed
            # by the deferred q passA units in qt1.
            xvg = [cp.tile([128, XVW], f16, name=f"xv{g}")
                   for g in range(4)]
            for g in range(4):
                nc.sync.dma_start(out=xvg[g][:],
                                  in_=xvR[:, g * XVW:(g + 1) * XVW])
            nc.sync.dma_start(out=xq_sb[:, XH:2 * XH],
                              in_=xqP[:, XH:2 * XH])

            def xk_v(d, c0, w):
                return xk_sb[:, d * 2048 + c0:d * 2048 + c0 + w]

            def xq_v(d, c0, w):
                h, c1 = divmod(c0, 1024)
                return xq_sb[:, h * XH + d * 1024 + c1:
                             h * XH + d * 1024 + c1 + w]

            # biases as f32 scalars (wa holds them as f16)
            bps = cp.tile([128, BPK], f32, name="bps")
            nc.vector.tensor_copy(out=bps[:], in_=wa[:, 0:BPK])
            bq_a, bq_b = bps[:, 0:1], bps[0:64, 1:2]
            bk_a, bk_b = bps[:, 2:3], bps[0:64, 3:4]
            bv_h = [bps[0:64, 4 + h:5 + h] for h in range(HPG)]

            # ---- PE warmup bridging the DMA wait ----
            junk = cp.tile([128, QT], f16, name="junk")
            nc.vector.memset(junk[:], 0.5)
            wupt = [ps_w.tile([128, QT], f32, name="W", tag="w")
                    for _ in range(2)]
            for i in range(24):
                nc.tensor.matmul(wupt[i % 2][:], lhsT=junk[:, 0:128],
                                 rhs=junk[:], start=True, stop=True)
            # dummy exp pulls the ACT exp-table load into the DMA shadow
            escr = cp.tile([128, BPK], f16, name="escr")
            nc.scalar.activation(escr[:], bps[:], Exp, scale=0.001)

            # ---- pre-attention projections, consuming chunk pairs as
            # they land: q passA cols 0:1024, then k passA (all cols) ----
            qT_a = cp.tile([128, S], f16, name="qT_a")
            qT_b = cp.tile([128, S], f16, name="qT_b")
            kT_a = cp.tile([128, S], f16, name="kT_a")
            kT_b = cp.tile([128, S], f16, name="kT_b")

            pjQ = ps_s.tile([128, 2 * QT], f32, name="S", tag="s")
            for d in range(DC):
                for n in range(2):
                    nc.tensor.matmul(
                        pjQ[:, n * QT:(n + 1) * QT],
                        lhsT=wq_sb[d][:, 0:128], rhs=xq_v(d, n * QT, QT),
                        start=(d == 0), stop=(d == DC - 1))
            for n in range(2):
                nc.vector.tensor_scalar_add(
                    qT_a[:, n * QT:(n + 1) * QT],
                    pjQ[:, n * QT:(n + 1) * QT], bq_a)

            pjK = [ps_s.tile([128, 2 * QT], f32, name="S", tag="s")
                   for _ in range(2)]
            for d in range(DC):
                for j2 in range(2):
                    for n in range(2):
                        nc.tensor.matmul(
                            pjK[j2][:, n * QT:(n + 1) * QT],
                            lhsT=wk_sb[d][:, 0:128],
                            rhs=xk_v(d, j2 * 1024 + n * QT, QT),
                            start=(d == 0), stop=(d == DC - 1))
            for j2 in range(2):
                for n in range(2):
                    cs = slice(j2 * 1024 + n * QT, j2 * 1024 + (n + 1) * QT)
                    nc.vector.tensor_scalar_add(
                        kT_a[:, cs], pjK[j2][:, n * QT:(n + 1) * QT], bk_a)

            # ---- deferred projection units ----
            fillers = []

            def add_unit(xv_fn, wsel, w_lo, w_hi, b, dst, c0, last, mirror):
                state = {}
                rows = w_hi - w_lo

                def mk(d):
                    def emit():
                        if d == 0:
                            state["pj"] = ps_w.tile([128, QT], f32,
                                                    name="W", tag="w")
                        nc.tensor.matmul(
                            state["pj"][0:rows, :],
                            lhsT=wsel[d][:, w_lo:w_hi],
                            rhs=xv_fn(d, c0, QT),
                            start=(d == 0), stop=(d == DC - 1))
                        if d == DC - 1:
                            nc.vector.tensor_scalar_add(
                                dst[0:rows, c0:c0 + QT],
                                state["pj"][0:rows, :], b)
                            if mirror:
                                nc.sync.dma_start(
                                    out=dst[64:128, c0:c0 + QT],
                                    in_=dst[0:64, c0:c0 + QT])
                    return emit
                for d in range(DC):
                    fillers.append(mk(d))

            for n in range(2):  # q passA cols 1024:2048
                add_unit(xq_v, wq_sb, 0, 128, bq_a, qT_a,
                         1024 + n * QT, False, False)
            for n4 in range(4):  # k passB
                add_unit(xk_v, wk_sb, 128, GW, bk_b, kT_b,
                         n4 * QT, n4 == 3, True)
            for n4 in range(4):  # q passB
                add_unit(xq_v, wq_sb, 128, GW, bq_b, qT_b,
                         n4 * QT, n4 == 3, True)

            # ---- v projection: just-in-time per seq tile inside qt0 ----
            v_sb = [None] * ST

            def v_proj(st):
                g4, s4 = st // 4, st % 4
                pv = ps_w.tile([128, QT], f32, name="W", tag="w")
                for d in range(DC):
                    c0 = (s4 * DC + d) * 128
                    nc.tensor.matmul(pv[:, 0:GW],
                                     lhsT=xvg[g4][:, c0:c0 + 128],
                                     rhs=wv_sb[d][:],
                                     start=(d == 0), stop=(d == DC - 1))
                vt = cp.tile([128, HPG, D_K + 1], f16, name=f"vsb{st}")
                nc.vector.tensor_copy(out=vt[:, :, 0:D_K],
                                      in_=pv[:, 0:GW].rearrange(
                                          "p (h w) -> p h w", h=HPG))
                nc.vector.memset(vt[:, :, D_K:D_K + 1], 1.0)
                v_sb[st] = vt

            # ---- attention state ----
            ctxT_a = [cp.tile([128, QT], f16, name=f"ctxTa{j}")
                      for j in range(NQT)]
            ctxT_b = [cp.tile([128, QT], f16, name=f"ctxTb{j}")
                      for j in range(NQT)]

            ones1 = cp.tile([1, D_K], f16, name="ones1")
            nc.vector.memset(ones1[:], 1.0)

            def normalize(C, h, qt):
                # Copy the accumulator (and denominator row) to SBUF first
                # so the PSUM bank frees early. h1's copy lands at
                # partitions 64:128 so the multiply's SBUF operands share a
                # start partition. The final (tail) normalize broadcasts the
                # reciprocal via a PE ones-matmul instead of GPSIMD — the PE
                # is idle there and it keeps the clock gate open.
                base = 64 if h == 1 else 0
                tail = h == 2 and qt == NQT - 1
                ctx_dst = (ctxT_a[qt][0:64] if h == 0 else
                           ctxT_a[qt][64:128] if h == 1 else
                           ctxT_b[qt][0:64])
                den = np_.tile([1, QT], f32, name="den", tag="den")
                if tail:
                    # scalar engine is idle after the last exp; keep the
                    # vector queue clear for the reciprocal chain
                    nc.scalar.copy(den[:], C[D_K:D_K + 1, :])
                else:
                    nc.vector.tensor_copy(out=den[:], in_=C[D_K:D_K + 1, :])
                Cc = np_.tile([128, QT], f32, name="Cc", tag="cc")
                nc.vector.tensor_copy(out=Cc[base:base + D_K, :],
                                      in_=C[0:D_K, :])
                r16 = np_.tile([1, QT], f16, name="r16", tag="r16")
                r = np_.tile([1, QT], f32, name="r", tag="r")
                nc.vector.reciprocal_approx_fast(out=r[:], in_=den[:])
                if tail:
                    nc.vector.tensor_copy(out=r16[:], in_=r[:])
                    bcp = ps_c.tile([128, QT], f32, name="C", tag="c")
                    nc.tensor.matmul(bcp[0:D_K, :], lhsT=ones1[:],
                                     rhs=r16[:], start=True, stop=True)
                    bc_ap = bcp[0:D_K, :]
                else:
                    bc = np_.tile([128, QT], f32, name="bc", tag="bc")
                    nc.gpsimd.partition_broadcast(bc[:], r[:])
                    bc_ap = bc[base:base + D_K, :]
                nc.vector.tensor_tensor(out=ctx_dst[:],
                                        in0=Cc[base:base + D_K, :],
                                        in1=bc_ap,
                                        op=mult)
                nc.vector.tensor_scalar_add(ctx_dst[:], ctx_dst[:], bv_h[h])
                if h == 2 and not tail:
                    # mirror so out_proj b-matmuls can alternate row groups
                    nc.sync.dma_start(out=ctxT_b[qt][64:128, :],
                                      in_=ctxT_b[qt][0:64, :])

            osb2 = [None]  # current [128, 1536] tile covering an st pair

            def ou_a(qt, u, po):
                st = u // 2
                ws = slice(st * 128, (st + 1) * 128)
                ns = slice((u % 2) * 384, (u % 2) * 384 + 384)
                nc.tensor.matmul(po[:], lhsT=ctxT_a[qt][:, ws],
                                 rhs=wo_a[:, ns], start=True, stop=False)

            def ou_b(qt, u, po, tail, sc=False):
                st = u // 2
                ws = slice(st * 128, (st + 1) * 128)
                ns = slice((u % 2) * 384, (u % 2) * 384 + 384)
                rb = (slice(0, 64) if (u % 2 == 0 or tail)
                      else slice(64, 128))
                nc.tensor.matmul(po[:], lhsT=ctxT_b[qt][rb, ws],
                                 rhs=wo_bm[rb, ns], start=False, stop=True)
                oc = (u % 4) * 384
                if sc or (tail and u % 2 == 1):
                    nc.scalar.copy(osb2[0][:, oc:oc + 384], po[:])
                else:
                    nc.vector.tensor_copy(out=osb2[0][:, oc:oc + 384],
                                          in_=po[:])
                if u % 4 == 3:
                    c0 = (qt * 4 + (u // 4) * 2) * D_MODEL
                    nc.sync.dma_start(out=out[:, c0:c0 + 2 * D_MODEL],
                                      in_=osb2[0][:])

            def out_unit(qt, u, po=None, tail=False, sc=False):
                if u % 4 == 0:
                    osb2[0] = op.tile([128, 2 * D_MODEL], f16, name="osb")
                if po is None:
                    po = ps_w.tile([128, QT], f32, name="W",
                                   tag="w")[:, 0:384]
                ou_a(qt, u, po)
                ou_b(qt, u, po, tail, sc)

            # ---- the flat attention stream ----
            # phase descriptors: ("h01", qt) x4 then ("h2", qt) x4
            steps = []
            for qt in range(NQT):
                steps += [("h01", qt, kc) for kc in range(KC)]
            for qt in range(NQT):
                steps += [("h2", qt, kc2) for kc2 in range(KC // 2)]

            Cs = {}   # (kind, qt) -> accumulator tile(s)
            S2q = [None] * len(steps)
            tail_po = []

            def emit_scores(i):
                kind, qt, kc = steps[i]
                S2 = ps_s.tile([128, 2 * QT], f32, name="S", tag="s")
                qs = slice(qt * QT, (qt + 1) * QT)
                if kind == "h01":
                    ks = slice(kc * 128, (kc + 1) * 128)
                    nc.tensor.matmul(S2[:, 0:QT], lhsT=kT_a[0:64, ks],
                                     rhs=qT_a[0:64, qs])
                    nc.tensor.matmul(S2[:, QT:2 * QT],
                                     lhsT=kT_a[64:128, ks],
                                     rhs=qT_a[64:128, qs])
                else:
                    for ii in (0, 1):
                        kcc = 2 * kc + ii
                        rg = slice(64 * ii, 64 * ii + 64)
                        nc.tensor.matmul(
                            S2[:, ii * QT:(ii + 1) * QT],
                            lhsT=kT_b[rg, kcc * 128:(kcc + 1) * 128],
                            rhs=qT_b[rg, qs])
                S2q[i] = S2

            emit_scores(0)
            for i, (kind, qt, kc) in enumerate(steps):
                e2 = ep.tile([128, 2 * QT], f16, name="expT")
                nc.scalar.activation(e2[:], S2q[i][:], Exp, scale=0.125)
                S2q[i] = None
                if i + 1 < len(steps):
                    emit_scores(i + 1)
                # PE-slack extras
                if kind == "h01":
                    if qt == 0:
                        v_proj(kc)
                    else:
                        rate = 2 if (qt == 1 and kc < 4) else 1
                        for _ in range(rate):
                            if fillers:
                                fillers.pop(0)()
                else:
                    if qt > 0:
                        # the last two injected units copy via the (by then
                        # idle) scalar engine so the vector queue is clear
                        # for the tail normalize
                        out_unit(qt - 1, kc,
                                 sc=(qt == NQT - 1 and kc >= 6))
                    elif fillers:
                        fillers.pop(0)()
                # ctx
                if kind == "h01":
                    if kc == 0:
                        Cs[qt] = {h: ps_c.tile([128, QT], f32, name="C",
                                               tag="c") for h in (0, 1)}
                    for h in (0, 1):
                        nc.tensor.matmul(Cs[qt][h][0:D_K + 1, :],
                                         lhsT=v_sb[kc][:, h, :],
                                         rhs=e2[:, h * QT:(h + 1) * QT],
                                         start=(kc == 0), stop=(kc == KC - 1))
                    if kc == KC - 1:
                        for h in (0, 1):
                            normalize(Cs[qt][h], h, qt)
                else:
                    if kc == 0:
                        Cs[("h2", qt)] = ps_c.tile([128, QT], f32,
                                                   name="C", tag="c")
                    C2 = Cs[("h2", qt)]
                    for ii in (0, 1):
                        kcc = 2 * kc + ii
                        nc.tensor.matmul(C2[0:D_K + 1, :],
                                         lhsT=v_sb[kcc][:, 2, :],
                                         rhs=e2[:, ii * QT:(ii + 1) * QT],
                                         start=(kcc == 0),
                                         stop=(kcc == KC - 1))
                    if kc == KC // 2 - 1:
                        if qt == NQT - 1:
                            # prestage the tail out_proj's ctxT_a halves:
                            # they only need ctxT_a, so they run during the
                            # normalize chain, keeping the PE warm. Borrow
                            # the idle score banks + w + c pools so seven
                            # accumulators are in flight.
                            tts = [ps_s.tile([128, 2 * QT], f32, name="S",
                                             tag="s") for _ in range(2)]
                            tail_po.extend([
                                tts[0][:, 0:384], tts[0][:, QT:QT + 384],
                                tts[1][:, 0:384], tts[1][:, QT:QT + 384]])
                            for _ in range(2):
                                tail_po.append(ps_w.tile(
                                    [128, QT], f32, name="W",
                                    tag="w")[:, 0:384])
                            tail_po.append(ps_c.tile(
                                [128, QT], f32, name="C", tag="c")[:, 0:384])
                            for u in range(7):
                                ou_a(NQT - 1, u, tail_po[u])
                        normalize(C2, 2, qt)

            # ---- tail: qt3's out_proj b-halves + copies + DMAs ----
            for u in range(4):
                if u == 0:
                    osb2[0] = op.tile([128, 2 * D_MODEL], f16, name="osb")
                ou_b(NQT - 1, u, tail_po[u], True)
            osb2[0] = op.tile([128, 2 * D_MODEL], f16, name="osb")
            ou_b(NQT - 1, 4, tail_po[4], True)
            ou_b(NQT - 1, 5, tail_po[5], True)
            p7 = ps_w.tile([128, QT], f32, name="W", tag="w")[:, 0:384]
            ou_a(NQT - 1, 7, p7)
            ou_b(NQT - 1, 6, tail_po[6], True)
            ou_b(NQT - 1, 7, p7, True)

    nc.compile()
    return nc


def _get_program():
    global _PROGRAM
    if _PROGRAM is None:
        _PROGRAM = _build_program()
    return _PROGRAM


def make_in_maps(query, key, value, Wq, bq, Wk, bk, Wv, bv, Wo, bo):
    """Build the 8 per-core input maps (host-side shard + pack + cast)."""
    q32 = np.asarray(query, np.float32)
    k32 = np.asarray(key, np.float32)
    v32 = np.asarray(value, np.float32)

    def pack_q(xT):
        # [768, 2048] -> [128, 2*6144]: halves x chunks x 1024
        return np.ascontiguousarray(
            xT.reshape(DC, 128, 2, 1024).transpose(2, 1, 0, 3)
        ).reshape(2, 128, XH).transpose(1, 0, 2).reshape(128, 2 * XH)

    def pack_k(xT):
        # [768, 2048] -> [128, 6*2048]: d-major full-width chunks
        return np.ascontiguousarray(
            xT.reshape(DC, 128, S).transpose(1, 0, 2)).reshape(128, DC * S)

    def pack_v(xT):
        # [768, 2048] -> [128, 16*768]: seq-tile-major
        return np.ascontiguousarray(
            xT.reshape(DC, 128, ST, 128).transpose(1, 2, 0, 3)
        ).reshape(128, ST * DC * 128)

    xP = {}
    for b in range(B):
        xP[b] = (pack_q(q32[b].T.astype(np.float16)),
                 pack_k(k32[b].T.astype(np.float16)),
                 pack_v(v32[b].T.astype(np.float16)))
    Wq = np.asarray(Wq, np.float32)
    Wk = np.asarray(Wk, np.float32)
    Wv = np.asarray(Wv, np.float32)
    Wo = np.asarray(Wo, np.float32)
    bq = np.asarray(bq, np.float32)
    bk = np.asarray(bk, np.float32)
    bv = np.asarray(bv, np.float32)
    in_maps = []
    for c in range(N_CORES):
        b, g = divmod(c, G)
        fs = slice(g * GW, (g + 1) * GW)
        xq, xk, xv = xP[b]
        wa = np.zeros((128, WA), np.float16)
        wa[:, 0] = bq[fs][0:128]
        wa[0:64, 1] = bq[fs][128:GW]
        wa[:, 2] = bk[fs][0:128]
        wa[0:64, 3] = bk[fs][128:GW]
        for h in range(HPG):
            wa[0:64, 4 + h] = bv[fs][h * 64:(h + 1) * 64]
        for i, W in enumerate((Wk, Wq)):
            Ws = W[:, fs]
            for d in range(DC):
                c0 = BPK + (i * DC + d) * GW
                wa[:, c0:c0 + GW] = Ws[d * 128:(d + 1) * 128, :].astype(
                    np.float16)
        wbp = np.zeros((128, WB), np.float16)
        Ws = Wv[:, fs]
        for d in range(DC):
            wbp[:, d * GW:(d + 1) * GW] = \
                Ws[d * 128:(d + 1) * 128, :].astype(np.float16)
        Wos = Wo[fs, :]
        wbp[:, DC * GW:DC * GW + D_MODEL] = Wos[0:128, :].astype(np.float16)
        wob = Wos[128:GW, :].astype(np.float16)
        wbp[0:64, DC * GW + D_MODEL:WB] = wob
        wbp[64:128, DC * GW + D_MODEL:WB] = wob
        in_maps.append({
            "xqP": xq, "xkP": xk, "xvR": xv,
            "wA": wa, "wB": wbp,
        })
    return in_maps


def unpack_out(o2):
    """[128, 16*768] partition-major partial -> [2048, 768]."""
    return np.asarray(o2, np.float32).reshape(
        128, ST, D_MODEL).transpose(1, 0, 2).reshape(S, D_MODEL)


def combine_outputs(results, bo):
    """Sum the per-core partial outputs into the full [B, S, D] output."""
    bo = np.asarray(bo, np.float32)
    out = np.zeros((B, S, D_MODEL), np.float32)
    for c in range(N_CORES):
        b = c // G
        out[b] += unpack_out(results[c]["out"])
    out += bo[None, None, :]
    return out


def kernel(**inputs):
    from concourse.bass_utils import run_bass_kernel_spmd

    nc = _get_program()
    in_maps = make_in_maps(**inputs)
    res = run_bass_kernel_spmd(nc, in_maps, list(range(N_CORES)))
    return combine_outputs(res.results, inputs["bo"])


# revision 34
# speedup vs baseline: 1.1899x; 1.0225x over previous
"""Multi-head attention (B=2, S=2048, D=768, H=12) on 8 trn2 NeuronCores.

Sharding: batch x head-group data/tensor parallel. Core c = b*4+g handles
batch b and heads [3g, 3g+3) (a 192-wide slice of the QKV projections and
the matching 192-row slice of Wo). Each core emits a partial [2048, 768]
fp16 output; the host sums the 4 head-group partials per batch and adds bo.

Device schedule. The kernel is dual-roofline (~100us PE streaming, ~97us
ACT exp). The DMA engines sustain ~330GB/s only with large contiguous
lines (they are descriptor-bound at ~10ns/partition-line), so inputs are
host-packed:
  wA  [128, 2312]   biases(f16) | wk | wq            (4.6KB lines)
  xqP [128, 12288]  column halves x chunks x 1024    (4KB-line pair xfers)
  xkP [128, 12288]  d-major chunks x 2048            (8KB-line pair xfers)
  wB  [128, 2688]   wv | wo_a | wo_b mirrored        (5.4KB lines)
  xvR [128, 12288]  seq-tile-quad groups             (6KB lines)
DMA order: wA, xq-half0 (3 chunk-pair transfers), xk (3 pair transfers),
wB, xv groups, xq-half1. Projections consume each pair as it lands; the
first exp fires ~23us in. Warmup junk matmuls bridge the DMA wait so the
PE HAM clock gate stays open.

Only k passA and q passA columns 0:1024 run before attention. The rest
(q passA cols 1024:2048, k/q passB) is deferred into attention PE slack
as six-matmul units cycling through the 2-bank "w" PSUM pool; the v
projection runs just-in-time per seq tile inside qt0.

The attention itself is ONE flat software-pipelined stream over phases
h01(qt0..3) then h2(qt0..3): at every step the NEXT step's score matmuls
are emitted before this step's ctx matmuls — across phase boundaries too
— so the scalar engine's monotonic semaphore wait for exp[i+1] never
covers ctx[i] and the exp stream never drains at a boundary. h2 phases
carry the previous q-tile's output projection, one (st, ns) unit per
iteration; out_proj(qt3) is the tail (borrowing idle score banks so four
units are in flight, copies alternating Scalar/Vector). normalize()
copies the accumulator to SBUF immediately so the PSUM bank frees early;
ctxT_b and wo_b are mirrored into partitions 64:127 so out_proj b-matmuls
alternate PE row groups.
"""

import numpy as np

D_MODEL = 768
NUM_HEADS = 12
D_K = 64
B = 2
S = 2048
N_CORES = 8
G = 4              # head groups (cores per batch)
GW = D_MODEL // G  # 192 features per group = 3 heads
HPG = 3            # heads per group
DC = D_MODEL // 128  # 6 d_model chunks
QT = 512           # q-tile width
NQT = S // QT      # 4
KC = S // 128      # 16 k chunks
ST = S // 128      # 16 seq tiles
BPK = 8            # packed bias columns
WA = BPK + 2 * DC * GW        # 2312: bias | wk | wq
WB = DC * GW + 2 * D_MODEL    # 2688: wv | wo_a | wo_b(mirrored)
XH = DC * 1024                # 6144: one xq half (6 chunks x 1024 seq)
XVW = 4 * DC * 128            # 3072: one xv group (4 seq tiles)

_PROGRAM = None


def _build_program():
    from concourse import bacc, tile
    import concourse.mybir as mybir

    f16 = mybir.dt.float16
    f32 = mybir.dt.float32
    Exp = mybir.ActivationFunctionType.Exp
    mult = mybir.AluOpType.mult

    nc = bacc.Bacc("TRN2", target_bir_lowering=False, debug=False,
                   enable_asserts=False)

    xkP = nc.dram_tensor("xkP", [128, DC * S], f16, kind="ExternalInput")
    xqP = nc.dram_tensor("xqP", [128, 2 * XH], f16, kind="ExternalInput")
    xvR = nc.dram_tensor("xvR", [128, 4 * XVW], f16, kind="ExternalInput")
    wA = nc.dram_tensor("wA", [128, WA], f16, kind="ExternalInput")
    wB = nc.dram_tensor("wB", [128, WB], f16, kind="ExternalInput")
    # partition-major output: out[p, st*768 + c] = result[st*128 + p, c].
    # One DMA per seq-tile PAIR with 3KB lines (the DMA engines are
    # descriptor-bound, so fewer/larger lines beat the row-major layout).
    out = nc.dram_tensor("out", [128, ST * D_MODEL], f16,
                         kind="ExternalOutput")

    with tile.TileContext(nc) as tc:
        with tc.tile_pool(name="const", bufs=1) as cp, \
             tc.tile_pool(name="expp", bufs=4) as ep, \
             tc.tile_pool(name="normp", bufs=2) as np_, \
             tc.tile_pool(name="outp", bufs=3) as op, \
             tc.tile_pool(name="ps_s", bufs=2, space="PSUM") as ps_s, \
             tc.tile_pool(name="ps_c", bufs=2, space="PSUM") as ps_c, \
             tc.tile_pool(name="ps_w", bufs=2, space="PSUM") as ps_w:

            # ---- DMA ----
            wa = cp.tile([128, WA], f16, name="wa")
            nc.sync.dma_start(out=wa[:], in_=wA[:])
            wk_sb = [wa[:, BPK + d * GW:BPK + (d + 1) * GW]
                     for d in range(DC)]
            wq_sb = [wa[:, BPK + DC * GW + d * GW:
                        BPK + DC * GW + (d + 1) * GW] for d in range(DC)]

            xq_sb = cp.tile([128, 2 * XH], f16, name="xq_sb")
            for p3 in range(3):
                nc.sync.dma_start(
                    out=xq_sb[:, p3 * 2048:(p3 + 1) * 2048],
                    in_=xqP[:, p3 * 2048:(p3 + 1) * 2048])
            xk_sb = cp.tile([128, DC * S], f16, name="xk_sb")
            for p3 in range(3):
                nc.sync.dma_start(
                    out=xk_sb[:, p3 * 4096:(p3 + 1) * 4096],
                    in_=xkP[:, p3 * 4096:(p3 + 1) * 4096])

            wb = cp.tile([128, WB], f16, name="wb")
            nc.sync.dma_start(out=wb[:], in_=wB[:])
            wv_sb = [wb[:, d * GW:(d + 1) * GW] for d in range(DC)]
            wo_a = wb[:, DC * GW:DC * GW + D_MODEL]
            wo_bm = wb[:, DC * GW + D_MODEL:WB]

            # all xv groups before xq-half1: qt0's just-in-time v projection
            # paces the early exp stream; xq cols 1024:2048 are only needed
            # by the deferred q passA units in qt1.
            xvg = [cp.tile([128, XVW], f16, name=f"xv{g}")
                   for g in range(4)]
            for g in range(4):
                nc.sync.dma_start(out=xvg[g][:],
                                  in_=xvR[:, g * XVW:(g + 1) * XVW])
            nc.sync.dma_start(out=xq_sb[:, XH:2 * XH],
                              in_=xqP[:, XH:2 * XH])

            def xk_v(d, c0, w):
                return xk_sb[:, d * 2048 + c0:d * 2048 + c0 + w]

            def xq_v(d, c0, w):
                h, c1 = divmod(c0, 1024)
                return xq_sb[:, h * XH + d * 1024 + c1:
                             h * XH + d * 1024 + c1 + w]

            # biases as f32 scalars (wa holds them as f16)
            bps = cp.tile([128, BPK], f32, name="bps")
            nc.vector.tensor_copy(out=bps[:], in_=wa[:, 0:BPK])
            bq_a, bq_b = bps[:, 0:1], bps[0:64, 1:2]
            bk_a, bk_b = bps[:, 2:3], bps[0:64, 3:4]
            bv_h = [bps[0:64, 4 + h:5 + h] for h in range(HPG)]

            # ---- PE warmup bridging the DMA wait ----
            junk = cp.tile([128, QT], f16, name="junk")
            nc.vector.memset(junk[:], 0.5)
            wupt = [ps_w.tile([128, QT], f32, name="W", tag="w")
                    for _ in range(2)]
            for i in range(24):
                nc.tensor.matmul(wupt[i % 2][:], lhsT=junk[:, 0:128],
                                 rhs=junk[:], start=True, stop=True)
            # dummy exp pulls the ACT exp-table load into the DMA shadow
            escr = cp.tile([128, BPK], f16, name="escr")
            nc.scalar.activation(escr[:], bps[:], Exp, scale=0.001)

            # ---- pre-attention projections, consuming chunk pairs as
            # they land: q passA cols 0:1024, then k passA (all cols) ----
            qT_a = cp.tile([128, S], f16, name="qT_a")
            qT_b = cp.tile([128, S], f16, name="qT_b")
            kT_a = cp.tile([128, S], f16, name="kT_a")
            kT_b = cp.tile([128, S], f16, name="kT_b")

            pjQ = ps_s.tile([128, 2 * QT], f32, name="S", tag="s")
            for d in range(DC):
                for n in range(2):
                    nc.tensor.matmul(
                        pjQ[:, n * QT:(n + 1) * QT],
                        lhsT=wq_sb[d][:, 0:128], rhs=xq_v(d, n * QT, QT),
                        start=(d == 0), stop=(d == DC - 1))
            for n in range(2):
                nc.vector.tensor_scalar_add(
                    qT_a[:, n * QT:(n + 1) * QT],
                    pjQ[:, n * QT:(n + 1) * QT], bq_a)

            pjK = [ps_s.tile([128, 2 * QT], f32, name="S", tag="s")
                   for _ in range(2)]
            for d in range(DC):
                for j2 in range(2):
                    for n in range(2):
                        nc.tensor.matmul(
                            pjK[j2][:, n * QT:(n + 1) * QT],
                            lhsT=wk_sb[d][:, 0:128],
                            rhs=xk_v(d, j2 * 1024 + n * QT, QT),
                            start=(d == 0), stop=(d == DC - 1))
            for j2 in range(2):
                for n in range(2):
                    cs = slice(j2 * 1024 + n * QT, j2 * 1024 + (n + 1) * QT)
                    nc.vector.tensor_scalar_add(
                        kT_a[:, cs], pjK[j2][:, n * QT:(n + 1) * QT], bk_a)

            # ---- deferred projection units ----
            fillers = []

            def add_unit(xv_fn, wsel, w_lo, w_hi, b, dst, c0, last, mirror):
                state = {}
                rows = w_hi - w_lo

                def mk(d):
                    def emit():
                        if d == 0:
                            state["pj"] = ps_w.tile([128, QT], f32,
                                                    name="W", tag="w")
                        nc.tensor.matmul(
                            state["pj"][0:rows, :],
                            lhsT=wsel[d][:, w_lo:w_hi],
                            rhs=xv_fn(d, c0, QT),
                            start=(d == 0), stop=(d == DC - 1))
                        if d == DC - 1:
                            nc.vector.tensor_scalar_add(
                                dst[0:rows, c0:c0 + QT],
                                state["pj"][0:rows, :], b)
                            if mirror:
                                nc.sync.dma_start(
                                    out=dst[64:128, c0:c0 + QT],
                                    in_=dst[0:64, c0:c0 + QT])
                    return emit
                for d in range(DC):
                    fillers.append(mk(d))

            for n in range(2):  # q passA cols 1024:2048
                add_unit(xq_v, wq_sb, 0, 128, bq_a, qT_a,
                         1024 + n * QT, False, False)
            for n4 in range(4):  # k passB
                add_unit(xk_v, wk_sb, 128, GW, bk_b, kT_b,
                         n4 * QT, n4 == 3, True)
            for n4 in range(4):  # q passB
                add_unit(xq_v, wq_sb, 128, GW, bq_b, qT_b,
                         n4 * QT, n4 == 3, True)

            # ---- v projection: just-in-time per seq tile inside qt0 ----
            v_sb = [None] * ST

            def v_proj(st):
                g4, s4 = st // 4, st % 4
                pv = ps_w.tile([128, QT], f32, name="W", tag="w")
                for d in range(DC):
                    c0 = (s4 * DC + d) * 128
                    nc.tensor.matmul(pv[:, 0:GW],
                                     lhsT=xvg[g4][:, c0:c0 + 128],
                                     rhs=wv_sb[d][:],
                                     start=(d == 0), stop=(d == DC - 1))
                vt = cp.tile([128, HPG, D_K + 1], f16, name=f"vsb{st}")
                nc.vector.tensor_copy(out=vt[:, :, 0:D_K],
                                      in_=pv[:, 0:GW].rearrange(
                                          "p (h w) -> p h w", h=HPG))
                nc.vector.memset(vt[:, :, D_K:D_K + 1], 1.0)
                v_sb[st] = vt

            # ---- attention state ----
            ctxT_a = [cp.tile([128, QT], f16, name=f"ctxTa{j}")
                      for j in range(NQT)]
            ctxT_b = [cp.tile([128, QT], f16, name=f"ctxTb{j}")
                      for j in range(NQT)]

            ones1 = cp.tile([1, D_K], f16, name="ones1")
            nc.vector.memset(ones1[:], 1.0)

            def normalize(C, h, qt):
                # Copy the accumulator (and denominator row) to SBUF first
                # so the PSUM bank frees early. h1's copy lands at
                # partitions 64:128 so the multiply's SBUF operands share a
                # start partition. The final (tail) normalize broadcasts the
                # reciprocal via a PE ones-matmul instead of GPSIMD — the PE
                # is idle there and it keeps the clock gate open.
                base = 64 if h == 1 else 0
                tail = h == 2 and qt == NQT - 1
                ctx_dst = (ctxT_a[qt][0:64] if h == 0 else
                           ctxT_a[qt][64:128] if h == 1 else
                           ctxT_b[qt][0:64])
                den = np_.tile([1, QT], f32, name="den", tag="den")
                nc.vector.tensor_copy(out=den[:], in_=C[D_K:D_K + 1, :])
                Cc = np_.tile([128, QT], f32, name="Cc", tag="cc")
                nc.vector.tensor_copy(out=Cc[base:base + D_K, :],
                                      in_=C[0:D_K, :])
                r16 = np_.tile([1, QT], f16, name="r16", tag="r16")
                r = np_.tile([1, QT], f32, name="r", tag="r")
                nc.vector.reciprocal_approx_fast(out=r[:], in_=den[:])
                if tail:
                    nc.vector.tensor_copy(out=r16[:], in_=r[:])
                    bcp = ps_c.tile([128, QT], f32, name="C", tag="c")
                    nc.tensor.matmul(bcp[0:D_K, :], lhsT=ones1[:],
                                     rhs=r16[:], start=True, stop=True)
                    bc_ap = bcp[0:D_K, :]
                else:
                    bc = np_.tile([128, QT], f32, name="bc", tag="bc")
                    nc.gpsimd.partition_broadcast(bc[:], r[:])
                    bc_ap = bc[base:base + D_K, :]
                nc.vector.tensor_tensor(out=ctx_dst[:],
                                        in0=Cc[base:base + D_K, :],
                                        in1=bc_ap,
                                        op=mult)
                nc.vector.tensor_scalar_add(ctx_dst[:], ctx_dst[:], bv_h[h])
                if h == 2 and not tail:
                    # mirror so out_proj b-matmuls can alternate row groups
                    nc.sync.dma_start(out=ctxT_b[qt][64:128, :],
                                      in_=ctxT_b[qt][0:64, :])

            osb2 = [None]  # current [128, 1536] tile covering an st pair

            def ou_a(qt, u, po):
                st = u // 2
                ws = slice(st * 128, (st + 1) * 128)
                ns = slice((u % 2) * 384, (u % 2) * 384 + 384)
                nc.tensor.matmul(po[:], lhsT=ctxT_a[qt][:, ws],
                                 rhs=wo_a[:, ns], start=True, stop=False)

            def ou_b(qt, u, po, tail, sc=False):
                st = u // 2
                ws = slice(st * 128, (st + 1) * 128)
                ns = slice((u % 2) * 384, (u % 2) * 384 + 384)
                rb = (slice(0, 64) if (u % 2 == 0 or tail)
                      else slice(64, 128))
                nc.tensor.matmul(po[:], lhsT=ctxT_b[qt][rb, ws],
                                 rhs=wo_bm[rb, ns], start=False, stop=True)
                oc = (u % 4) * 384
                if sc or (tail and u % 2 == 1):
                    nc.scalar.copy(osb2[0][:, oc:oc + 384], po[:])
                else:
                    nc.vector.tensor_copy(out=osb2[0][:, oc:oc + 384],
                                          in_=po[:])
                if u % 4 == 3:
                    c0 = (qt * 4 + (u // 4) * 2) * D_MODEL
                    nc.sync.dma_start(out=out[:, c0:c0 + 2 * D_MODEL],
                                      in_=osb2[0][:])

            def out_unit(qt, u, po=None, tail=False, sc=False):
                if u % 4 == 0:
                    osb2[0] = op.tile([128, 2 * D_MODEL], f16, name="osb")
                if po is None:
                    po = ps_w.tile([128, QT], f32, name="W",
                                   tag="w")[:, 0:384]
                ou_a(qt, u, po)
                ou_b(qt, u, po, tail, sc)

            # ---- the flat attention stream ----
            # phase descriptors: ("h01", qt) x4 then ("h2", qt) x4
            steps = []
            for qt in range(NQT):
                steps += [("h01", qt, kc) for kc in range(KC)]
            for qt in range(NQT):
                steps += [("h2", qt, kc2) for kc2 in range(KC // 2)]

            Cs = {}   # (kind, qt) -> accumulator tile(s)
            S2q = [None] * len(steps)
            tail_po = []

            def emit_scores(i):
                kind, qt, kc = steps[i]
                S2 = ps_s.tile([128, 2 * QT], f32, name="S", tag="s")
                qs = slice(qt * QT, (qt + 1) * QT)
                if kind == "h01":
                    ks = slice(kc * 128, (kc + 1) * 128)
                    nc.tensor.matmul(S2[:, 0:QT], lhsT=kT_a[0:64, ks],
                                     rhs=qT_a[0:64, qs])
                    nc.tensor.matmul(S2[:, QT:2 * QT],
                                     lhsT=kT_a[64:128, ks],
                                     rhs=qT_a[64:128, qs])
                else:
                    for ii in (0, 1):
                        kcc = 2 * kc + ii
                        rg = slice(64 * ii, 64 * ii + 64)
                        nc.tensor.matmul(
                            S2[:, ii * QT:(ii + 1) * QT],
                            lhsT=kT_b[rg, kcc * 128:(kcc + 1) * 128],
                            rhs=qT_b[rg, qs])
                S2q[i] = S2

            emit_scores(0)
            for i, (kind, qt, kc) in enumerate(steps):
                e2 = ep.tile([128, 2 * QT], f16, name="expT")
                nc.scalar.activation(e2[:], S2q[i][:], Exp, scale=0.125)
                S2q[i] = None
                if i + 1 < len(steps):
                    emit_scores(i + 1)
                # PE-slack extras
                if kind == "h01":
                    if qt == 0:
                        v_proj(kc)
                    else:
                        rate = 2 if (qt == 1 and kc < 4) else 1
                        for _ in range(rate):
                            if fillers:
                                fillers.pop(0)()
                else:
                    if qt > 0:
                        # the last two injected units copy via the (by then
                        # idle) scalar engine so the vector queue is clear
                        # for the tail normalize
                        out_unit(qt - 1, kc,
                                 sc=(qt == NQT - 1 and kc >= 6))
                    elif fillers:
                        fillers.pop(0)()
                # ctx
                if kind == "h01":
                    if kc == 0:
                        Cs[qt] = {h: ps_c.tile([128, QT], f32, name="C",
                                               tag="c") for h in (0, 1)}
                    for h in (0, 1):
                        nc.tensor.matmul(Cs[qt][h][0:D_K + 1, :],
                                         lhsT=v_sb[kc][:, h, :],
                                         rhs=e2[:, h * QT:(h + 1) * QT],
                                         start=(kc == 0), stop=(kc == KC - 1))
                    if kc == KC - 1:
                        for h in (0, 1):
                            normalize(Cs[qt][h], h, qt)
                else:
                    if kc == 0:
                        Cs[("h2", qt)] = ps_c.tile([128, QT], f32,
                                                   name="C", tag="c")
                    C2 = Cs[("h2", qt)]
                    for ii in (0, 1):
                        kcc = 2 * kc + ii
                        nc.tensor.matmul(C2[0:D_K + 1, :],
                                         lhsT=v_sb[kcc][:, 2, :],
                                         rhs=e2[:, ii * QT:(ii + 1) * QT],
                                         start=(kcc == 0),
                                         stop=(kcc == KC - 1))
                    if kc == KC // 2 - 1:
                        if qt == NQT - 1:
                            # prestage the tail out_proj's ctxT_a halves:
                            # they only need ctxT_a, so they run during the
                            # normalize chain, keeping the PE warm. Borrow
                            # the idle score banks + w + c pools so seven
                            # accumulators are in flight.
                            tts = [ps_s.tile([128, 2 * QT], f32, name="S",
                                             tag="s") for _ in range(2)]
                            tail_po.extend([
                                tts[0][:, 0:384], tts[0][:, QT:QT + 384],
                                tts[1][:, 0:384], tts[1][:, QT:QT + 384]])
                            for _ in range(2):
                                tail_po.append(ps_w.tile(
                                    [128, QT], f32, name="W",
                                    tag="w")[:, 0:384])
                            tail_po.append(ps_c.tile(
                                [128, QT], f32, name="C", tag="c")[:, 0:384])
                            for u in range(7):
                                ou_a(NQT - 1, u, tail_po[u])
                        normalize(C2, 2, qt)

            # ---- tail: qt3's out_proj b-halves + copies + DMAs ----
            for u in range(4):
                if u == 0:
                    osb2[0] = op.tile([128, 2 * D_MODEL], f16, name="osb")
                ou_b(NQT - 1, u, tail_po[u], True)
            osb2[0] = op.tile([128, 2 * D_MODEL], f16, name="osb")
            ou_b(NQT - 1, 4, tail_po[4], True)
            ou_b(NQT - 1, 5, tail_po[5], True)
            p7 = ps_w.tile([128, QT], f32, name="W", tag="w")[:, 0:384]
            ou_a(NQT - 1, 7, p7)
            ou_b(NQT - 1, 6, tail_po[6], True)
            ou_b(NQT - 1, 7, p7, True)

    nc.compile()
    return nc


def _get_program():
    global _PROGRAM
    if _PROGRAM is None:
        _PROGRAM = _build_program()
    return _PROGRAM


def make_in_maps(query, key, value, Wq, bq, Wk, bk, Wv, bv, Wo, bo):
    """Build the 8 per-core input maps (host-side shard + pack + cast)."""
    q32 = np.asarray(query, np.float32)
    k32 = np.asarray(key, np.float32)
    v32 = np.asarray(value, np.float32)

    def pack_q(xT):
        # [768, 2048] -> [128, 2*6144]: halves x chunks x 1024
        return np.ascontiguousarray(
            xT.reshape(DC, 128, 2, 1024).transpose(2, 1, 0, 3)
        ).reshape(2, 128, XH).transpose(1, 0, 2).reshape(128, 2 * XH)

    def pack_k(xT):
        # [768, 2048] -> [128, 6*2048]: d-major full-width chunks
        return np.ascontiguousarray(
            xT.reshape(DC, 128, S).transpose(1, 0, 2)).reshape(128, DC * S)

    def pack_v(xT):
        # [768, 2048] -> [128, 16*768]: seq-tile-major
        return np.ascontiguousarray(
            xT.reshape(DC, 128, ST, 128).transpose(1, 2, 0, 3)
        ).reshape(128, ST * DC * 128)

    xP = {}
    for b in range(B):
        xP[b] = (pack_q(q32[b].T.astype(np.float16)),
                 pack_k(k32[b].T.astype(np.float16)),
                 pack_v(v32[b].T.astype(np.float16)))
    Wq = np.asarray(Wq, np.float32)
    Wk = np.asarray(Wk, np.float32)
    Wv = np.asarray(Wv, np.float32)
    Wo = np.asarray(Wo, np.float32)
    bq = np.asarray(bq, np.float32)
    bk = np.asarray(bk, np.float32)
    bv = np.asarray(bv, np.float32)
    in_maps = []
    for c in range(N_CORES):
        b, g = divmod(c, G)
        fs = slice(g * GW, (g + 1) * GW)
        xq, xk, xv = xP[b]
        wa = np.zeros((128, WA), np.float16)
        wa[:, 0] = bq[fs][0:128]
        wa[0:64, 1] = bq[fs][128:GW]
        wa[:, 2] = bk[fs][0:128]
        wa[0:64, 3] = bk[fs][128:GW]
        for h in range(HPG):
            wa[0:64, 4 + h] = bv[fs][h * 64:(h + 1) * 64]
        for i, W in enumerate((Wk, Wq)):
            Ws = W[:, fs]
            for d in range(DC):
                c0 = BPK + (i * DC + d) * GW
                wa[:, c0:c0 + GW] = Ws[d * 128:(d + 1) * 128, :].astype(
                    np.float16)
        wbp = np.zeros((128, WB), np.float16)
        Ws = Wv[:, fs]
        for d in range(DC):
            wbp[:, d * GW:(d + 1) * GW] = \
                Ws[d * 128:(d + 1) * 128, :].astype(np.float16)
        Wos = Wo[fs, :]
        wbp[:, DC * GW:DC * GW + D_MODEL] = Wos[0:128, :].astype(np.float16)
        wob = Wos[128:GW, :].astype(np.float16)
        wbp[0:64, DC * GW + D_MODEL:WB] = wob
        wbp[64:128, DC * GW + D_MODEL:WB] = wob
        in_maps.append({
            "xqP": xq, "xkP": xk, "xvR": xv,
            "wA": wa, "wB": wbp,
        })
    return in_maps


def unpack_out(o2):
    """[128, 16*768] partition-major partial -> [2048, 768]."""
    return np.asarray(o2, np.float32).reshape(
        128, ST, D_MODEL).transpose(1, 0, 2).reshape(S, D_MODEL)


def combine_outputs(results, bo):
    """Sum the per-core partial outputs into the full [B, S, D] output."""
    bo = np.asarray(bo, np.float32)
    out = np.zeros((B, S, D_MODEL), np.float32)
    for c in range(N_CORES):
        b = c // G
        out[b] += unpack_out(results[c]["out"])
    out += bo[None, None, :]
    return out


def kernel(**inputs):
    from concourse.bass_utils import run_bass_kernel_spmd

    nc = _get_program()
    in_maps = make_in_maps(**inputs)
    res = run_bass_kernel_spmd(nc, in_maps, list(range(N_CORES)))
    return combine_outputs(res.results, inputs["bo"])
